# revision 13
# baseline (speedup 1.0000x reference)
"""Trainium2 Bass kernel for nn_CanineAttention (block-diagonal local attention).

Reference computation (per batch b):
  q/k/v = x @ W{q,k,v} + b{q,k,v}            x: [B,S,H]=[4,4096,768]
  per chunk of W=128 tokens, per head (NH=12, HD=64):
    scores = q k^T / 8 + (1-mask_diag)*(-1e4); probs = softmax(scores)
    ctx = probs @ v
  out = LayerNorm(ctx @ Wo + bo + x) * gamma + beta

Sharding: tokens (B*S = 16384) are split contiguously across 8 cores
(2048 tokens = 16 chunks per core; chunk boundaries align), fully
data-parallel, no collectives.

Device layout notes:
  - Projections run with the contraction dim (h_in) on partitions, so the
    kernel consumes x TRANSPOSED (host pre-transposes to [H, tokens]).
  - qT, kT are produced in [h_out, t] layout (head dims on partitions),
    v in natural [t, h_out] layout; attention produces ctxT [h_out, t]
    which feeds the output projection as lhsT directly.
  - Softmax skips max-subtraction (scores are O(1) here; the additive mask
    bias is <= 0 so exp() only underflows, never overflows).
  - Precision modes:
      "f32"    : native fp32 matmuls (4 cycles/row on PE)
      "bf16"   : single bf16 pass (1 cycle/row)
      "split3" : x=hi+lo, W=hi+lo in bf16; q = xh@Wh + xh@Wl + xl@Wh
                 (3 cycles/row, ~fp32 accuracy)
    Attention internals (scores/softmax/PV) are always fp32.
"""

import numpy as np
import ml_dtypes
from contextlib import ExitStack

import concourse.bass as bass
import concourse.tile as tile
from concourse import bacc, mybir
from concourse.bass_utils import run_bass_kernel_spmd
from concourse.masks import make_identity

# ---------------- problem constants (hardcoded per contract) ----------------
B, S, H, NH, W = 4, 4096, 768, 12, 128
HD = H // NH            # 64
C = S // W              # 32 chunks
NEG = -10000.0
EPS = 1e-12

NCORES = 8
TPC = B * S // NCORES   # 2048 tokens per core
CPC = TPC // W          # 16 chunks per core
BLK = 512               # tokens per processing block
NBLK = TPC // BLK       # 4 blocks
CPB = BLK // W          # 4 chunks per block
NG = H // 128           # 6 partition-chunks over H
NPAIR = NH // 2         # 6 head pairs (2 heads x 64 dims = 128 partitions)

F32 = mybir.dt.float32
BF16 = mybir.dt.bfloat16
FP = mybir.ActivationFunctionType
OP = mybir.AluOpType
AX = mybir.AxisListType

MODE = "split3"   # default precision mode; see module docstring


def _bf16(a):
    return a.astype(ml_dtypes.bfloat16)


# ---------------------------------------------------------------------------
# kernel builder
# ---------------------------------------------------------------------------

def _build(mode, use_mask, use_qbias, use_kbias, use_vbias, use_ln_affine, reps=1):
    """Build + compile the SPMD single-core program. Returns (nc, input_names).

    reps>1 repeats the whole computation (idempotent output writes) so HW
    kernel time can be measured as the slope over reps."""
    nc = bacc.Bacc(
        "TRN2", target_bir_lowering=False, debug=False,
        enable_asserts=False, num_devices=NCORES,
    )

    split = (mode == "split3")
    wdt = F32 if mode == "f32" else BF16
    xdt = wdt

    # ---------------- DRAM I/O ----------------
    names = []

    def dram_in(name, shape, dt):
        names.append(name)
        return nc.dram_tensor(name, shape, dt, kind="ExternalInput").ap()

    xt_hi = dram_in("xt_hi", [H, TPC], xdt)            # x^T (hi part if split)
    xt_lo = dram_in("xt_lo", [H, TPC], xdt) if split else None
    xres = dram_in("xres", [TPC, H], F32)              # x + bo (residual input)
    w_dram = {"wq": dram_in("wq_hi", [H, H], wdt),
              "wk": dram_in("wk_hi", [H, H], wdt),
              "wv": dram_in("wv_hi", [H, H], wdt),
              "wo": dram_in("wo_hi", [H, H], wdt)}
    if split:
        for wn in ("wq", "wk", "wv", "wo"):
            w_dram[wn + "_lo"] = dram_in(wn + "_lo", [H, H], wdt)
    bq = dram_in("bq", [128, NG], F32) if use_qbias else None   # (bq/8) chunked
    bk = dram_in("bk", [128, NG], F32) if use_kbias else None
    bvb = dram_in("bvb", [128, H], F32) if use_vbias else None  # bv broadcast
    gmb = dram_in("gmb", [128, H], F32) if use_ln_affine else None
    btb = dram_in("btb", [128, H], F32) if use_ln_affine else None
    mbias = dram_in("mbias", [CPC, W, W], F32) if use_mask else None
    out = nc.dram_tensor("out", [TPC, H], F32, kind="ExternalOutput").ap()

    # matmul pass list: (x-half, weight-key)
    if split:
        passes = {wn: [("hi", wn), ("hi", wn + "_lo"), ("lo", wn)]
                  for wn in ("wq", "wk", "wv", "wo")}
    else:
        passes = {wn: [("hi", wn)] for wn in ("wq", "wk", "wv", "wo")}

    with tile.TileContext(nc) as tc, ExitStack() as ctx:
        const = ctx.enter_context(tc.tile_pool(name="const", bufs=1))
        xp = ctx.enter_context(tc.tile_pool(name="xp", bufs=2))
        qkv = ctx.enter_context(tc.tile_pool(name="qkv", bufs=1))
        attn = ctx.enter_context(tc.tile_pool(name="attn", bufs=3))
        ctxp = ctx.enter_context(tc.tile_pool(name="ctxp", bufs=1))
        outp = ctx.enter_context(tc.tile_pool(name="outp", bufs=2))
        pproj = ctx.enter_context(tc.tile_pool(name="pproj", bufs=2, space="PSUM"))
        psc = ctx.enter_context(tc.tile_pool(name="psc", bufs=2, space="PSUM"))
        ppt = ctx.enter_context(tc.tile_pool(name="ppt", bufs=2, space="PSUM"))
        pcx = ctx.enter_context(tc.tile_pool(name="pcx", bufs=2, space="PSUM"))

        # ---------------- constants ----------------
        ident = const.tile([128, 128], F32, tag="ident")
        make_identity(nc, ident)

        w_sb = {}   # key -> list of NG chunk tiles [128, H]
        for wn, ap in w_dram.items():
            w_sb[wn] = []
            for g in range(NG):
                t = const.tile([128, H], wdt, tag=f"{wn}{g}")
                nc.sync.dma_start(t[:], ap[g * 128:(g + 1) * 128, :])
                w_sb[wn].append(t)

        bq_sb = bk_sb = bvb_sb = gmb_sb = btb_sb = None
        if use_qbias:
            bq_sb = const.tile([128, NG], F32, tag="bq")
            nc.sync.dma_start(bq_sb[:], bq)
        if use_kbias:
            bk_sb = const.tile([128, NG], F32, tag="bk")
            nc.sync.dma_start(bk_sb[:], bk)
        if use_vbias:
            bvb_sb = const.tile([128, H], F32, tag="bvb")
            nc.sync.dma_start(bvb_sb[:], bvb)
        if use_ln_affine:
            gmb_sb = const.tile([128, H], F32, tag="gmb")
            nc.sync.dma_start(gmb_sb[:], gmb)
            btb_sb = const.tile([128, H], F32, tag="btb")
            nc.sync.dma_start(btb_sb[:], btb)

        # ---------------- per token-block ----------------
        # reps>1: device-side hardware loop repeating the whole computation
        # (for slope-based HW timing); body is identical each iteration.
        import contextlib
        rep_cm = tc.For_i(0, reps, 1) if reps > 1 else contextlib.nullcontext()
        with rep_cm:
          for blk in range(NBLK):
            t0 = blk * BLK

            # -- load x^T block --
            xth = [xp.tile([128, BLK], xdt, tag=f"xth{g}", name=f"xth{g}") for g in range(NG)]
            for g in range(NG):
                nc.sync.dma_start(xth[g][:], xt_hi[g * 128:(g + 1) * 128, t0:t0 + BLK])
            if split:
                xtl = [xp.tile([128, BLK], xdt, tag=f"xtl{g}", name=f"xtl{g}") for g in range(NG)]
                for g in range(NG):
                    nc.sync.dma_start(xtl[g][:], xt_lo[g * 128:(g + 1) * 128, t0:t0 + BLK])

            def xop(sel, g):
                return xth[g] if sel == "hi" else xtl[g]

            # -- Q/K projections (transposed layout [h_out, t]) --
            qT, kT = [], []
            for which, wn, dst in (("q", "wq", qT), ("k", "wk", kT)):
                for go in range(NG):
                    ps = pproj.tile([128, BLK], F32, tag="proj")
                    mms = [(wkey, gi, xsel) for xsel, wkey in passes[wn] for gi in range(NG)]
                    for i, (wkey, gi, xsel) in enumerate(mms):
                        nc.tensor.matmul(
                            ps[:],
                            w_sb[wkey][gi][:, go * 128:(go + 1) * 128],
                            xop(xsel, gi)[:],
                            start=(i == 0), stop=(i == len(mms) - 1),
                        )
                    sb = qkv.tile([128, BLK], F32, tag=f"{which}T{go}")
                    scl = 0.125 if which == "q" else 1.0
                    has_b = use_qbias if which == "q" else use_kbias
                    if has_b:
                        bias = (bq_sb if which == "q" else bk_sb)[:, go:go + 1]
                        nc.scalar.activation(sb[:], ps[:], FP.Identity, bias=bias, scale=scl)
                    else:
                        nc.scalar.activation(sb[:], ps[:], FP.Copy, scale=scl)
                    dst.append(sb)

            # -- V projection (natural layout [t, h_out]) --
            vN = []
            for tt in range(CPB):
                vt = qkv.tile([128, H], F32, tag=f"v{tt}")
                for nhalf in range(2):
                    n0 = nhalf * 384
                    ps = pproj.tile([128, 384], F32, tag="proj")
                    mms = [(wkey, gi, xsel) for xsel, wkey in passes["wv"] for gi in range(NG)]
                    for i, (wkey, gi, xsel) in enumerate(mms):
                        nc.tensor.matmul(
                            ps[:],
                            xop(xsel, gi)[:, tt * 128:(tt + 1) * 128],
                            w_sb[wkey][gi][:, n0:n0 + 384],
                            start=(i == 0), stop=(i == len(mms) - 1),
                        )
                    if use_vbias:
                        nc.vector.tensor_add(vt[:, n0:n0 + 384], ps[:], bvb_sb[:, n0:n0 + 384])
                    else:
                        nc.vector.tensor_copy(vt[:, n0:n0 + 384], ps[:])
                vN.append(vt)

            # -- block-diagonal attention --
            # ctxT tiles [128, BLK] per partition-chunk (= head pair)
            cxdt = F32 if mode == "f32" else BF16
            cxh = [ctxp.tile([128, BLK], cxdt, tag=f"cxh{g}", name=f"cxh{g}") for g in range(NG)]
            cxl = [ctxp.tile([128, BLK], BF16, tag=f"cxl{g}", name=f"cxl{g}") for g in range(NG)] if split else None

            for cc in range(CPB):
                ts = slice(cc * 128, (cc + 1) * 128)
                chunk_idx = blk * CPB + cc
                if use_mask:
                    mb = attn.tile([128, W], F32, tag="mb")
                    nc.sync.dma_start(mb[:], mbias[chunk_idx])
                for g in range(NPAIR):
                    # scores for head pair (2g, 2g+1); row-tiled concurrent
                    # matmuls MUST land in separate PSUM banks (same-bank
                    # concurrent writes are a HW collision).
                    ps_s = [psc.tile([128, W], F32, tag="sc", name=f"scps{h}")
                            for h in range(2)]
                    for h in range(2):
                        p0 = h * 64
                        nc.tensor.matmul(
                            ps_s[h][:],
                            qT[g][p0:p0 + 64, ts],
                            kT[g][p0:p0 + 64, ts],
                            start=True, stop=True,
                            tile_position=(p0, 0),
                        )
                    den = attn.tile([128, 2], F32, tag="den")
                    ex = attn.tile([128, 2 * W], F32, tag="ex")
                    for h in range(2):
                        src = ps_s[h][:]
                        if use_mask:
                            sm = attn.tile([128, W], F32, tag="sm")
                            nc.vector.tensor_add(sm[:], src, mb[:])
                            src = sm[:]
                        nc.scalar.activation(
                            ex[:, h * W:(h + 1) * W], src, FP.Exp,
                            accum_out=den[:, h:h + 1],
                        )
                    rec = attn.tile([128, 2], F32, tag="rec")
                    nc.vector.reciprocal(rec[:], den[:])
                    # normalize, then transpose each head's probs on the PE
                    pr = attn.tile([128, 2 * W], F32, tag="pr")
                    for h in range(2):
                        nc.vector.tensor_scalar_mul(
                            pr[:, h * W:(h + 1) * W], ex[:, h * W:(h + 1) * W],
                            rec[:, h:h + 1],
                        )
                    ps_t = ppt.tile([128, 2 * W], F32, tag="pt")
                    for h in range(2):
                        nc.tensor.matmul(
                            ps_t[:, h * W:(h + 1) * W], pr[:, h * W:(h + 1) * W],
                            ident[:], is_transpose=True,
                            skip_group_check=(h == 1),
                        )
                    pts = attn.tile([128, 2 * W], F32, tag="pts")
                    nc.vector.tensor_copy(pts[:], ps_t[:])
                    # PV: ctxT pair [128 (2 heads x 64 dims), 128 tokens]
                    ps_c = pcx.tile([128, W], F32, tag="cx")
                    for h in range(2):
                        hd0 = (2 * g + h) * HD
                        nc.tensor.matmul(
                            ps_c[h * 64:(h + 1) * 64, :],
                            vN[cc][:, hd0:hd0 + HD],
                            pts[:, h * W:(h + 1) * W],
                            start=True, stop=True,
                            tile_position=(0, h * 64),
                            skip_group_check=(h == 1),
                        )
                    if split:
                        nc.scalar.activation(cxh[g][:, ts], ps_c[:], FP.Copy)
                        nc.vector.tensor_sub(cxl[g][:, ts], ps_c[:], cxh[g][:, ts])
                    else:
                        nc.vector.tensor_copy(cxh[g][:, ts], ps_c[:])  # casts if bf16

            # -- output projection + residual + LayerNorm --
            if split:
                opasses = [(cxh, "wo"), (cxh, "wo_lo"), (cxl, "wo")]
            else:
                opasses = [(cxh, "wo")]
            for tt in range(CPB):
                r0 = t0 + tt * 128
                xr = outp.tile([128, H], F32, tag="xr")
                nc.sync.dma_start(xr[:], xres[r0:r0 + 128, :])
                hsb = outp.tile([128, H], F32, tag="hsb")
                for nhalf in range(2):
                    n0 = nhalf * 384
                    ps = pproj.tile([128, 384], F32, tag="proj")
                    mms = [(cx, wkey, gi) for cx, wkey in opasses for gi in range(NG)]
                    for i, (cx, wkey, gi) in enumerate(mms):
                        nc.tensor.matmul(
                            ps[:],
                            cx[gi][:, tt * 128:(tt + 1) * 128],
                            w_sb[wkey][gi][:, n0:n0 + 384],
                            start=(i == 0), stop=(i == len(mms) - 1),
                        )
                    nc.vector.tensor_add(hsb[:, n0:n0 + 384], ps[:], xr[:, n0:n0 + 384])

                # LayerNorm over the free dim (H)
                s1 = outp.tile([128, 1], F32, tag="s1")
                nc.vector.reduce_sum(s1[:], hsb[:], axis=AX.X)
                nmu = outp.tile([128, 1], F32, tag="nmu")
                nc.vector.tensor_scalar_mul(nmu[:], s1[:], -1.0 / H)
                xc = outp.tile([128, H], F32, tag="xc")
                nc.scalar.activation(xc[:], hsb[:], FP.Identity, bias=nmu[:])
                # var = mean(xc^2) + EPS, via ACT Square with row-accumulate
                # (tensor_tensor_reduce crashes on HW)
                sq = outp.tile([128, H], F32, tag="sq")
                s2 = outp.tile([128, 1], F32, tag="s2")
                nc.scalar.activation(sq[:], xc[:], FP.Square, accum_out=s2[:])
                var1 = outp.tile([128, 1], F32, tag="var1")
                nc.vector.tensor_scalar(var1[:], s2[:], 1.0 / H, EPS, op0=OP.mult, op1=OP.add)
                # rstd = 1/sqrt(var): bit-trick seed + 3 Newton steps (on DVE,
                # avoiding the ACT sqrt table-set switch and its poor ULP)
                rstd = outp.tile([128, 1], F32, tag="rstd")
                t1 = outp.tile([128, 1], F32, tag="t1n")
                ri = rstd[:].bitcast(mybir.dt.int32)
                nc.vector.tensor_scalar(
                    ri, var1[:].bitcast(mybir.dt.int32), 1, None,
                    op0=OP.logical_shift_right,
                )
                nc.vector.tensor_scalar(ri, ri, -1, 0x5F3759DF, op0=OP.mult, op1=OP.add)
                for _ in range(3):
                    nc.vector.tensor_mul(t1[:], rstd[:], rstd[:])
                    nc.vector.tensor_mul(t1[:], t1[:], var1[:])
                    nc.vector.tensor_scalar(t1[:], t1[:], -0.5, 1.5, op0=OP.mult, op1=OP.add)
                    nc.vector.tensor_mul(rstd[:], rstd[:], t1[:])
                ot = outp.tile([128, H], F32, tag="ot")
                nc.vector.tensor_scalar_mul(ot[:], xc[:], rstd[:])
                if use_ln_affine:
                    nc.vector.tensor_mul(ot[:], ot[:], gmb_sb[:])
                    nc.vector.tensor_add(ot[:], ot[:], btb_sb[:])
                nc.sync.dma_start(out[r0:r0 + 128, :], ot[:])

    nc.compile()
    return nc, names


# ---------------------------------------------------------------------------
# host-side wrapper
# ---------------------------------------------------------------------------

_CACHE = {}


def _get_program(mode, use_mask, use_qbias, use_kbias, use_vbias, use_ln_affine, reps=1):
    key = (mode, use_mask, use_qbias, use_kbias, use_vbias, use_ln_affine, reps)
    if key not in _CACHE:
        _CACHE[key] = _build(*key[:-1], reps=reps)
    return _CACHE[key]


def _prep_inputs(inputs, mode):
    """Host preprocessing -> per-core in_maps + program flags."""
    hs = np.ascontiguousarray(np.asarray(inputs["hidden_states"], dtype=np.float32))
    mask = np.asarray(inputs["attention_mask"], dtype=np.float32)
    Wq = np.asarray(inputs["Wq"], np.float32); bq = np.asarray(inputs["bq"], np.float32)
    Wk = np.asarray(inputs["Wk"], np.float32); bk = np.asarray(inputs["bk"], np.float32)
    Wv = np.asarray(inputs["Wv"], np.float32); bv = np.asarray(inputs["bv"], np.float32)
    Wo = np.asarray(inputs["Wo"], np.float32); bo = np.asarray(inputs["bo"], np.float32)
    gm = np.asarray(inputs["ln_gamma"], np.float32)
    bt = np.asarray(inputs["ln_beta"], np.float32)

    split = (mode == "split3")
    use_mask = not np.all(mask == 1.0)
    use_qbias = bool(np.any(bq)); use_kbias = bool(np.any(bk))
    use_vbias = bool(np.any(bv))
    use_ln_affine = bool(np.any(gm != 1.0) or np.any(bt))

    x = hs.reshape(B * S, H)
    xres_full = x + bo[None, :] if np.any(bo) else x

    def wpack(w):
        if mode == "f32":
            return {"hi": np.ascontiguousarray(w)}
        wh = _bf16(w)
        d = {"hi": np.ascontiguousarray(wh)}
        if split:
            d["lo"] = np.ascontiguousarray(_bf16(w - wh.astype(np.float32)))
        return d

    wq, wk, wv, wo = wpack(Wq), wpack(Wk), wpack(Wv), wpack(Wo)

    if use_mask:
        # per-core diagonal [W,W] blocks of the mask -> additive bias
        m4 = mask.reshape(B, C, W, C, W)
        idx = np.arange(C)
        mblk = m4[:, idx, :, idx, :]                 # [C,B,W,W]
        mblk = np.transpose(mblk, (1, 0, 2, 3))      # [B,C,W,W]
        bias_blocks = ((1.0 - mblk) * NEG).astype(np.float32).reshape(B * C, W, W)

    in_maps = []
    for c in range(NCORES):
        sl = x[c * TPC:(c + 1) * TPC]                # [TPC, H]
        m = {}
        if mode == "f32":
            m["xt_hi"] = np.ascontiguousarray(sl.T)
        else:
            xh = _bf16(sl)
            m["xt_hi"] = np.ascontiguousarray(xh.T)
            if split:
                m["xt_lo"] = np.ascontiguousarray(_bf16(sl - xh.astype(np.float32)).T)
        m["xres"] = np.ascontiguousarray(xres_full[c * TPC:(c + 1) * TPC])
        for wn, d in (("wq", wq), ("wk", wk), ("wv", wv), ("wo", wo)):
            m[wn + "_hi"] = d["hi"]
            if split:
                m[wn + "_lo"] = d["lo"]
        if use_qbias:
            m["bq"] = np.ascontiguousarray((bq / 8.0).reshape(NG, 128).T)
        if use_kbias:
            m["bk"] = np.ascontiguousarray(bk.reshape(NG, 128).T)
        if use_vbias:
            m["bvb"] = np.ascontiguousarray(np.broadcast_to(bv, (128, H)))
        if use_ln_affine:
            m["gmb"] = np.ascontiguousarray(np.broadcast_to(gm, (128, H)))
            m["btb"] = np.ascontiguousarray(np.broadcast_to(bt, (128, H)))
        if use_mask:
            m["mbias"] = np.ascontiguousarray(bias_blocks[c * CPC:(c + 1) * CPC])
        in_maps.append(m)

    flags = (use_mask, use_qbias, use_kbias, use_vbias, use_ln_affine)
    return in_maps, flags


def run(inputs, mode=None, trace=False, reps=1):
    """Run the kernel; returns (output [B,S,H] f32, BassKernelResults)."""
    mode = mode or MODE
    in_maps, flags = _prep_inputs(inputs, mode)
    nc, names = _get_program(mode, *flags, reps=reps)
    in_maps = [{k: v for k, v in m.items() if k in names} for m in in_maps]
    res = run_bass_kernel_spmd(nc, in_maps, list(range(NCORES)), trace=trace)
    outs = [res.results[c]["out"] for c in range(NCORES)]
    full = np.concatenate(outs, axis=0).reshape(B, S, H).astype(np.float32)
    return full, res


def kernel(**inputs):
    out, _ = run(inputs)
    return out


# revision 14
# speedup vs baseline: 1.7784x; 1.7784x over previous
"""Trainium2 Bass kernel for nn_CanineAttention (block-diagonal local attention).

Reference computation (per batch b):
  q/k/v = x @ W{q,k,v} + b{q,k,v}            x: [B,S,H]=[4,4096,768]
  per chunk of W=128 tokens, per head (NH=12, HD=64):
    scores = q k^T / 8 + (1-mask_diag)*(-1e4); probs = softmax(scores)
    ctx = probs @ v
  out = LayerNorm(ctx @ Wo + bo + x) * gamma + beta

Sharding: tokens (B*S = 16384) are split contiguously across 8 cores
(2048 tokens = 16 chunks per core; chunk boundaries align), fully
data-parallel, no collectives.

Device layout notes:
  - Projections run with the contraction dim (h_in) on partitions, so the
    kernel consumes x TRANSPOSED (host pre-transposes to [H, tokens]).
  - qT, kT are produced in [h_out, t] layout (head dims on partitions),
    v in natural [t, h_out] layout; attention produces ctxT [h_out, t]
    which feeds the output projection as lhsT directly.
  - Softmax skips max-subtraction (scores are O(1) here; the additive mask
    bias is <= 0 so exp() only underflows, never overflows).
  - Precision modes:
      "f32"    : native fp32 matmuls (4 cycles/row on PE)
      "bf16"   : single bf16 pass (1 cycle/row)
      "split3" : x=hi+lo, W=hi+lo in bf16; q = xh@Wh + xh@Wl + xl@Wh
                 (3 cycles/row, ~fp32 accuracy)
    Attention internals (scores/softmax/PV) are always fp32.
"""

import numpy as np
import ml_dtypes
from contextlib import ExitStack

import concourse.bass as bass
import concourse.tile as tile
from concourse import bacc, mybir
from concourse.bass_utils import run_bass_kernel_spmd
from concourse.masks import make_identity

# ---------------- problem constants (hardcoded per contract) ----------------
B, S, H, NH, W = 4, 4096, 768, 12, 128
HD = H // NH            # 64
C = S // W              # 32 chunks
NEG = -10000.0
EPS = 1e-12

NCORES = 8
TPC = B * S // NCORES   # 2048 tokens per core
CPC = TPC // W          # 16 chunks per core
BLK = 512               # tokens per processing block
NBLK = TPC // BLK       # 4 blocks
CPB = BLK // W          # 4 chunks per block
NG = H // 128           # 6 partition-chunks over H
NPAIR = NH // 2         # 6 head pairs (2 heads x 64 dims = 128 partitions)

F32 = mybir.dt.float32
BF16 = mybir.dt.bfloat16
FP = mybir.ActivationFunctionType
OP = mybir.AluOpType
AX = mybir.AxisListType

MODE = "split3"   # default precision mode; see module docstring


def _bf16(a):
    return a.astype(ml_dtypes.bfloat16)


# ---------------------------------------------------------------------------
# kernel builder
# ---------------------------------------------------------------------------

def _build(mode, use_mask, use_qbias, use_kbias, use_vbias, use_ln_affine, reps=1):
    """Build + compile the SPMD single-core program. Returns (nc, input_names).

    reps>1 repeats the whole computation (idempotent output writes) so HW
    kernel time can be measured as the slope over reps."""
    nc = bacc.Bacc(
        "TRN2", target_bir_lowering=False, debug=False,
        enable_asserts=False, num_devices=NCORES,
    )

    split = (mode == "split3")
    wdt = F32 if mode == "f32" else BF16
    xdt = wdt

    # ---------------- DRAM I/O ----------------
    names = []

    def dram_in(name, shape, dt):
        names.append(name)
        return nc.dram_tensor(name, shape, dt, kind="ExternalInput").ap()

    xt_hi = dram_in("xt_hi", [H, TPC], xdt)            # x^T (hi part if split)
    xt_lo = dram_in("xt_lo", [H, TPC], xdt) if split else None
    xres = dram_in("xres", [TPC, H], F32)              # x + bo (residual input)
    w_dram = {"wq": dram_in("wq_hi", [H, H], wdt),
              "wk": dram_in("wk_hi", [H, H], wdt),
              "wv": dram_in("wv_hi", [H, H], wdt),
              "wo": dram_in("wo_hi", [H, H], wdt)}
    if split:
        for wn in ("wq", "wk", "wv", "wo"):
            w_dram[wn + "_lo"] = dram_in(wn + "_lo", [H, H], wdt)
    bq = dram_in("bq", [128, NG], F32) if use_qbias else None   # (bq/8) chunked
    bk = dram_in("bk", [128, NG], F32) if use_kbias else None
    bvb = dram_in("bvb", [128, H], F32) if use_vbias else None  # bv broadcast
    gmb = dram_in("gmb", [128, H], F32) if use_ln_affine else None
    btb = dram_in("btb", [128, H], F32) if use_ln_affine else None
    mbias = dram_in("mbias", [CPC, W, W], F32) if use_mask else None
    out = nc.dram_tensor("out", [TPC, H], F32, kind="ExternalOutput").ap()

    # matmul pass list: (x-half, weight-key)
    if split:
        passes = {wn: [("hi", wn), ("hi", wn + "_lo"), ("lo", wn)]
                  for wn in ("wq", "wk", "wv", "wo")}
    else:
        passes = {wn: [("hi", wn)] for wn in ("wq", "wk", "wv", "wo")}

    with tile.TileContext(nc) as tc, ExitStack() as ctx:
        const = ctx.enter_context(tc.tile_pool(name="const", bufs=1))
        xp = ctx.enter_context(tc.tile_pool(name="xp", bufs=2))
        qkv = ctx.enter_context(tc.tile_pool(name="qkv", bufs=1))
        attn = ctx.enter_context(tc.tile_pool(name="attn", bufs=3))
        ctxp = ctx.enter_context(tc.tile_pool(name="ctxp", bufs=1))
        outp = ctx.enter_context(tc.tile_pool(name="outp", bufs=2))
        pproj = ctx.enter_context(tc.tile_pool(name="pproj", bufs=2, space="PSUM"))
        psc = ctx.enter_context(tc.tile_pool(name="psc", bufs=2, space="PSUM"))
        ppt = ctx.enter_context(tc.tile_pool(name="ppt", bufs=2, space="PSUM"))
        pcx = ctx.enter_context(tc.tile_pool(name="pcx", bufs=2, space="PSUM"))

        # ---------------- constants ----------------
        ident = const.tile([128, 128], F32, tag="ident")
        make_identity(nc, ident)

        w_sb = {}   # key -> list of NG chunk tiles [128, H]
        for wn, ap in w_dram.items():
            w_sb[wn] = []
            for g in range(NG):
                t = const.tile([128, H], wdt, tag=f"{wn}{g}")
                nc.sync.dma_start(t[:], ap[g * 128:(g + 1) * 128, :])
                w_sb[wn].append(t)

        bq_sb = bk_sb = bvb_sb = gmb_sb = btb_sb = None
        if use_qbias:
            bq_sb = const.tile([128, NG], F32, tag="bq")
            nc.sync.dma_start(bq_sb[:], bq)
        if use_kbias:
            bk_sb = const.tile([128, NG], F32, tag="bk")
            nc.sync.dma_start(bk_sb[:], bk)
        if use_vbias:
            bvb_sb = const.tile([128, H], F32, tag="bvb")
            nc.sync.dma_start(bvb_sb[:], bvb)
        if use_ln_affine:
            gmb_sb = const.tile([128, H], F32, tag="gmb")
            nc.sync.dma_start(gmb_sb[:], gmb)
            btb_sb = const.tile([128, H], F32, tag="btb")
            nc.sync.dma_start(btb_sb[:], btb)

        # ---------------- per token-block ----------------
        # reps>1: device-side hardware loop repeating the whole computation
        # (for slope-based HW timing); body is identical each iteration.
        import contextlib
        rep_cm = tc.For_i(0, reps, 1) if reps > 1 else contextlib.nullcontext()
        with rep_cm:
          for blk in range(NBLK):
            t0 = blk * BLK

            # -- load x^T block --
            xth = [xp.tile([128, BLK], xdt, tag=f"xth{g}", name=f"xth{g}") for g in range(NG)]
            for g in range(NG):
                nc.sync.dma_start(xth[g][:], xt_hi[g * 128:(g + 1) * 128, t0:t0 + BLK])
            if split:
                xtl = [xp.tile([128, BLK], xdt, tag=f"xtl{g}", name=f"xtl{g}") for g in range(NG)]
                for g in range(NG):
                    nc.sync.dma_start(xtl[g][:], xt_lo[g * 128:(g + 1) * 128, t0:t0 + BLK])

            def xop(sel, g):
                return xth[g] if sel == "hi" else xtl[g]

            # -- Q/K projections (transposed layout [h_out, t]) --
            qT, kT = [], []
            for which, wn, dst in (("q", "wq", qT), ("k", "wk", kT)):
                for go in range(NG):
                    ps = pproj.tile([128, BLK], F32, tag="proj")
                    mms = [(wkey, gi, xsel) for xsel, wkey in passes[wn] for gi in range(NG)]
                    for i, (wkey, gi, xsel) in enumerate(mms):
                        nc.tensor.matmul(
                            ps[:],
                            w_sb[wkey][gi][:, go * 128:(go + 1) * 128],
                            xop(xsel, gi)[:],
                            start=(i == 0), stop=(i == len(mms) - 1),
                        )
                    sb = qkv.tile([128, BLK], F32, tag=f"{which}T{go}")
                    scl = 0.125 if which == "q" else 1.0
                    has_b = use_qbias if which == "q" else use_kbias
                    if has_b:
                        bias = (bq_sb if which == "q" else bk_sb)[:, go:go + 1]
                        nc.scalar.activation(sb[:], ps[:], FP.Identity, bias=bias, scale=scl)
                    else:
                        nc.scalar.activation(sb[:], ps[:], FP.Copy, scale=scl)
                    dst.append(sb)

            # -- V projection (natural layout [t, h_out]) --
            vN = []
            for tt in range(CPB):
                vt = qkv.tile([128, H], F32, tag=f"v{tt}")
                for nhalf in range(2):
                    n0 = nhalf * 384
                    ps = pproj.tile([128, 384], F32, tag="proj")
                    mms = [(wkey, gi, xsel) for xsel, wkey in passes["wv"] for gi in range(NG)]
                    for i, (wkey, gi, xsel) in enumerate(mms):
                        nc.tensor.matmul(
                            ps[:],
                            xop(xsel, gi)[:, tt * 128:(tt + 1) * 128],
                            w_sb[wkey][gi][:, n0:n0 + 384],
                            start=(i == 0), stop=(i == len(mms) - 1),
                        )
                    if use_vbias:
                        nc.vector.tensor_add(vt[:, n0:n0 + 384], ps[:], bvb_sb[:, n0:n0 + 384])
                    else:
                        nc.vector.tensor_copy(vt[:, n0:n0 + 384], ps[:])
                vN.append(vt)

            # -- block-diagonal attention --
            # ctxT tiles [128, BLK] per partition-chunk (= head pair)
            cxdt = F32 if mode == "f32" else BF16
            cxh = [ctxp.tile([128, BLK], cxdt, tag=f"cxh{g}", name=f"cxh{g}") for g in range(NG)]
            cxl = [ctxp.tile([128, BLK], BF16, tag=f"cxl{g}", name=f"cxl{g}") for g in range(NG)] if split else None

            for cc in range(CPB):
                ts = slice(cc * 128, (cc + 1) * 128)
                chunk_idx = blk * CPB + cc
                if use_mask:
                    mb = attn.tile([128, W], F32, tag="mb")
                    nc.sync.dma_start(mb[:], mbias[chunk_idx])
                for g in range(NPAIR):
                    # scores for head pair (2g, 2g+1); row-tiled concurrent
                    # matmuls MUST land in separate PSUM banks (same-bank
                    # concurrent writes are a HW collision).
                    ps_s = [psc.tile([128, W], F32, tag="sc", name=f"scps{h}")
                            for h in range(2)]
                    for h in range(2):
                        p0 = h * 64
                        nc.tensor.matmul(
                            ps_s[h][:],
                            qT[g][p0:p0 + 64, ts],
                            kT[g][p0:p0 + 64, ts],
                            start=True, stop=True,
                            tile_position=(p0, 0),
                        )
                    den = attn.tile([128, 2], F32, tag="den")
                    ex = attn.tile([128, 2 * W], F32, tag="ex")
                    for h in range(2):
                        src = ps_s[h][:]
                        if use_mask:
                            sm = attn.tile([128, W], F32, tag="sm")
                            nc.vector.tensor_add(sm[:], src, mb[:])
                            src = sm[:]
                        nc.scalar.activation(
                            ex[:, h * W:(h + 1) * W], src, FP.Exp,
                            accum_out=den[:, h:h + 1],
                        )
                    rec = attn.tile([128, 2], F32, tag="rec")
                    nc.vector.reciprocal(rec[:], den[:])
                    # normalize, then transpose each head's probs on the PE
                    pr = attn.tile([128, 2 * W], F32, tag="pr")
                    for h in range(2):
                        nc.vector.tensor_scalar_mul(
                            pr[:, h * W:(h + 1) * W], ex[:, h * W:(h + 1) * W],
                            rec[:, h:h + 1],
                        )
                    ps_t = ppt.tile([128, 2 * W], F32, tag="pt")
                    for h in range(2):
                        nc.tensor.matmul(
                            ps_t[:, h * W:(h + 1) * W], pr[:, h * W:(h + 1) * W],
                            ident[:], is_transpose=True,
                            skip_group_check=(h == 1),
                        )
                    pts = attn.tile([128, 2 * W], F32, tag="pts")
                    nc.vector.tensor_copy(pts[:], ps_t[:])
                    # PV: ctxT pair [128 (2 heads x 64 dims), 128 tokens]
                    ps_c = pcx.tile([128, W], F32, tag="cx")
                    for h in range(2):
                        hd0 = (2 * g + h) * HD
                        nc.tensor.matmul(
                            ps_c[h * 64:(h + 1) * 64, :],
                            vN[cc][:, hd0:hd0 + HD],
                            pts[:, h * W:(h + 1) * W],
                            start=True, stop=True,
                            tile_position=(0, h * 64),
                            skip_group_check=(h == 1),
                        )
                    if split:
                        nc.scalar.activation(cxh[g][:, ts], ps_c[:], FP.Copy)
                        nc.vector.tensor_sub(cxl[g][:, ts], ps_c[:], cxh[g][:, ts])
                    else:
                        nc.vector.tensor_copy(cxh[g][:, ts], ps_c[:])  # casts if bf16

            # -- output projection + residual + LayerNorm --
            if split:
                opasses = [(cxh, "wo"), (cxh, "wo_lo"), (cxl, "wo")]
            else:
                opasses = [(cxh, "wo")]
            for tt in range(CPB):
                r0 = t0 + tt * 128
                xr = outp.tile([128, H], F32, tag="xr")
                nc.sync.dma_start(xr[:], xres[r0:r0 + 128, :])
                hsb = outp.tile([128, H], F32, tag="hsb")
                for nhalf in range(2):
                    n0 = nhalf * 384
                    ps = pproj.tile([128, 384], F32, tag="proj")
                    mms = [(cx, wkey, gi) for cx, wkey in opasses for gi in range(NG)]
                    for i, (cx, wkey, gi) in enumerate(mms):
                        nc.tensor.matmul(
                            ps[:],
                            cx[gi][:, tt * 128:(tt + 1) * 128],
                            w_sb[wkey][gi][:, n0:n0 + 384],
                            start=(i == 0), stop=(i == len(mms) - 1),
                        )
                    nc.vector.tensor_add(hsb[:, n0:n0 + 384], ps[:], xr[:, n0:n0 + 384])

                # LayerNorm over the free dim (H)
                s1 = outp.tile([128, 1], F32, tag="s1")
                nc.vector.reduce_sum(s1[:], hsb[:], axis=AX.X)
                nmu = outp.tile([128, 1], F32, tag="nmu")
                nc.vector.tensor_scalar_mul(nmu[:], s1[:], -1.0 / H)
                xc = outp.tile([128, H], F32, tag="xc")
                nc.scalar.activation(xc[:], hsb[:], FP.Identity, bias=nmu[:])
                # var = mean(xc^2) + EPS on DVE (tensor_tensor_reduce crashes
                # on HW; ACT Square risks an Exp<->Square table-set switch)
                sq = outp.tile([128, H], F32, tag="sq")
                nc.vector.tensor_mul(sq[:], xc[:], xc[:])
                s2 = outp.tile([128, 1], F32, tag="s2")
                nc.vector.reduce_sum(s2[:], sq[:], axis=AX.X)
                var1 = outp.tile([128, 1], F32, tag="var1")
                nc.vector.tensor_scalar(var1[:], s2[:], 1.0 / H, EPS, op0=OP.mult, op1=OP.add)
                # rstd = 1/sqrt(var): bit-trick seed + 3 Newton steps (on DVE,
                # avoiding the ACT sqrt table-set switch and its poor ULP)
                rstd = outp.tile([128, 1], F32, tag="rstd")
                t1 = outp.tile([128, 1], F32, tag="t1n")
                ri = rstd[:].bitcast(mybir.dt.int32)
                nc.vector.tensor_scalar(
                    ri, var1[:].bitcast(mybir.dt.int32), 1, None,
                    op0=OP.logical_shift_right,
                )
                nc.vector.tensor_scalar(ri, ri, -1, 0x5F3759DF, op0=OP.mult, op1=OP.add)
                for _ in range(3):
                    nc.vector.tensor_mul(t1[:], rstd[:], rstd[:])
                    nc.vector.tensor_mul(t1[:], t1[:], var1[:])
                    nc.vector.tensor_scalar(t1[:], t1[:], -0.5, 1.5, op0=OP.mult, op1=OP.add)
                    nc.vector.tensor_mul(rstd[:], rstd[:], t1[:])
                ot = outp.tile([128, H], F32, tag="ot")
                nc.vector.tensor_scalar_mul(ot[:], xc[:], rstd[:])
                if use_ln_affine:
                    nc.vector.tensor_mul(ot[:], ot[:], gmb_sb[:])
                    nc.vector.tensor_add(ot[:], ot[:], btb_sb[:])
                nc.sync.dma_start(out[r0:r0 + 128, :], ot[:])

    nc.compile()
    return nc, names


# ---------------------------------------------------------------------------
# host-side wrapper
# ---------------------------------------------------------------------------

_CACHE = {}


def _get_program(mode, use_mask, use_qbias, use_kbias, use_vbias, use_ln_affine, reps=1):
    key = (mode, use_mask, use_qbias, use_kbias, use_vbias, use_ln_affine, reps)
    if key not in _CACHE:
        _CACHE[key] = _build(*key[:-1], reps=reps)
    return _CACHE[key]


def _prep_inputs(inputs, mode):
    """Host preprocessing -> per-core in_maps + program flags."""
    hs = np.ascontiguousarray(np.asarray(inputs["hidden_states"], dtype=np.float32))
    mask = np.asarray(inputs["attention_mask"], dtype=np.float32)
    Wq = np.asarray(inputs["Wq"], np.float32); bq = np.asarray(inputs["bq"], np.float32)
    Wk = np.asarray(inputs["Wk"], np.float32); bk = np.asarray(inputs["bk"], np.float32)
    Wv = np.asarray(inputs["Wv"], np.float32); bv = np.asarray(inputs["bv"], np.float32)
    Wo = np.asarray(inputs["Wo"], np.float32); bo = np.asarray(inputs["bo"], np.float32)
    gm = np.asarray(inputs["ln_gamma"], np.float32)
    bt = np.asarray(inputs["ln_beta"], np.float32)

    split = (mode == "split3")
    use_mask = not np.all(mask == 1.0)
    use_qbias = bool(np.any(bq)); use_kbias = bool(np.any(bk))
    use_vbias = bool(np.any(bv))
    use_ln_affine = bool(np.any(gm != 1.0) or np.any(bt))

    x = hs.reshape(B * S, H)
    xres_full = x + bo[None, :] if np.any(bo) else x

    def wpack(w):
        if mode == "f32":
            return {"hi": np.ascontiguousarray(w)}
        wh = _bf16(w)
        d = {"hi": np.ascontiguousarray(wh)}
        if split:
            d["lo"] = np.ascontiguousarray(_bf16(w - wh.astype(np.float32)))
        return d

    wq, wk, wv, wo = wpack(Wq), wpack(Wk), wpack(Wv), wpack(Wo)

    if use_mask:
        # per-core diagonal [W,W] blocks of the mask -> additive bias
        m4 = mask.reshape(B, C, W, C, W)
        idx = np.arange(C)
        mblk = m4[:, idx, :, idx, :]                 # [C,B,W,W]
        mblk = np.transpose(mblk, (1, 0, 2, 3))      # [B,C,W,W]
        bias_blocks = ((1.0 - mblk) * NEG).astype(np.float32).reshape(B * C, W, W)

    in_maps = []
    for c in range(NCORES):
        sl = x[c * TPC:(c + 1) * TPC]                # [TPC, H]
        m = {}
        if mode == "f32":
            m["xt_hi"] = np.ascontiguousarray(sl.T)
        else:
            xh = _bf16(sl)
            m["xt_hi"] = np.ascontiguousarray(xh.T)
            if split:
                m["xt_lo"] = np.ascontiguousarray(_bf16(sl - xh.astype(np.float32)).T)
        m["xres"] = np.ascontiguousarray(xres_full[c * TPC:(c + 1) * TPC])
        for wn, d in (("wq", wq), ("wk", wk), ("wv", wv), ("wo", wo)):
            m[wn + "_hi"] = d["hi"]
            if split:
                m[wn + "_lo"] = d["lo"]
        if use_qbias:
            m["bq"] = np.ascontiguousarray((bq / 8.0).reshape(NG, 128).T)
        if use_kbias:
            m["bk"] = np.ascontiguousarray(bk.reshape(NG, 128).T)
        if use_vbias:
            m["bvb"] = np.ascontiguousarray(np.broadcast_to(bv, (128, H)))
        if use_ln_affine:
            m["gmb"] = np.ascontiguousarray(np.broadcast_to(gm, (128, H)))
            m["btb"] = np.ascontiguousarray(np.broadcast_to(bt, (128, H)))
        if use_mask:
            m["mbias"] = np.ascontiguousarray(bias_blocks[c * CPC:(c + 1) * CPC])
        in_maps.append(m)

    flags = (use_mask, use_qbias, use_kbias, use_vbias, use_ln_affine)
    return in_maps, flags


def run(inputs, mode=None, trace=False, reps=1):
    """Run the kernel; returns (output [B,S,H] f32, BassKernelResults)."""
    mode = mode or MODE
    in_maps, flags = _prep_inputs(inputs, mode)
    nc, names = _get_program(mode, *flags, reps=reps)
    in_maps = [{k: v for k, v in m.items() if k in names} for m in in_maps]
    res = run_bass_kernel_spmd(nc, in_maps, list(range(NCORES)), trace=trace)
    outs = [res.results[c]["out"] for c in range(NCORES)]
    full = np.concatenate(outs, axis=0).reshape(B, S, H).astype(np.float32)
    return full, res


def kernel(**inputs):
    out, _ = run(inputs)
    return out
